# revision 1
# baseline (speedup 1.0000x reference)
"""MoE (Dariush) layer for Trainium2, 8 NeuronCores, expert-parallel.

Strategy
--------
The reference computes every expert densely ([B,S,E,D]) and then keeps only
the top-2 experts per token.  We instead:

  1. (host) run the router exactly as the reference does (logits + fixed
     gumbel noise + softmax + top-2)  -- tiny compute, bit-stable.
  2. (host) gather each expert's assigned tokens into a padded, transposed
     bf16 buffer; experts are sharded 2-per-core across the 8 cores
     (expert parallelism per the sharding hint).
  3. (device, SPMD bass/tile kernel) for each expert slot run the SwiGLU
     FFN on just its tokens: h1 = x@W1+b1, h2 = x@W2+b2,
     y = (silu(h1)*h2)@Wout+bout, all matmuls in bf16 with fp32 PSUM
     accumulation, everything transposed ([D, tokens] layout) so no on-chip
     transposes are needed.
  4. (host) scatter-combine y with the top-2 gates into the full output.

Only ~2/16 of the reference FLOPs run, and the kernel is near the
memory roofline (weights + gathered activations per core).
"""

import sys

for _p in ("/opt/trn_rl_repo", "/root/.axon_site/_ro/trn_rl_repo"):
    if _p not in sys.path:
        sys.path.insert(0, _p)

from contextlib import ExitStack

import ml_dtypes
import numpy as np

TOP_K = 2
NOISE_SCALE = 0.05
P = 128  # partitions
N_CORES = 8

_PROGRAM_CACHE = {}
_NOISE_CACHE = {}


def _gumbel_noise(shape):
    """Reproduce jax.random.gumbel(jax.random.key(42), shape, f32) on CPU."""
    key = (tuple(shape),)
    if key not in _NOISE_CACHE:
        import jax

        cpu = jax.devices("cpu")[0]
        with jax.default_device(cpu):
            n = jax.random.gumbel(jax.random.key(42), shape, "float32")
            _NOISE_CACHE[key] = np.asarray(n)
    return _NOISE_CACHE[key]


def _pick_blocks(cmax):
    """Pick (NB, NBLK): NB blocks of NBLK columns, NBLK <= 512, minimizing
    estimated PE time NB * (NBLK/4.8 + 27ns)."""
    best = None
    for nb in range(1, 65):
        nblk = -(-cmax // nb)  # ceil
        nblk = -(-nblk // 16) * 16  # round up to 16
        if nblk > 512:
            continue
        cost = nb * (nblk / 4.8 + 27.0)
        if best is None or cost < best[0]:
            best = (cost, nb, nblk)
    assert best is not None
    return best[1], best[2]


def _build_program(NB, NBLK, KT, MT, DH, has_bias):
    import concourse.bass as bass
    import concourse.tile as tile
    from concourse import bacc, mybir

    C = NB * NBLK
    BF16 = mybir.dt.bfloat16
    F32 = mybir.dt.float32
    Silu = mybir.ActivationFunctionType.Silu
    Ident = mybir.ActivationFunctionType.Identity

    nc = bacc.Bacc(
        "TRN2", target_bir_lowering=False, debug=False, num_devices=N_CORES
    )
    xt = nc.dram_tensor("xt", [2, KT, P, C], BF16, kind="ExternalInput").ap()
    w1 = nc.dram_tensor("w1", [2, KT, P, DH], BF16, kind="ExternalInput").ap()
    w2 = nc.dram_tensor("w2", [2, KT, P, DH], BF16, kind="ExternalInput").ap()
    wo = nc.dram_tensor("wo", [2, KT, P, DH], BF16, kind="ExternalInput").ap()
    if has_bias:
        bb = nc.dram_tensor("bb", [3, 2, P, MT], F32, kind="ExternalInput").ap()
    yt = nc.dram_tensor("yt", [2, MT, P, C], F32, kind="ExternalOutput").ap()

    with tile.TileContext(nc) as tc, ExitStack() as ctx:
        wpool = ctx.enter_context(tc.tile_pool(name="w", bufs=1))
        xpool = ctx.enter_context(tc.tile_pool(name="xp", bufs=1))
        spool = ctx.enter_context(tc.tile_pool(name="sp", bufs=3))
        upool = ctx.enter_context(tc.tile_pool(name="up", bufs=2))
        ypool = ctx.enter_context(tc.tile_pool(name="yp", bufs=4))
        pspool = ctx.enter_context(tc.tile_pool(name="ps", bufs=2, space="PSUM"))

        wt = {}
        xts = {}
        for s in range(2):
            for k in range(KT):
                for nm, src in (("w1", w1), ("w2", w2), ("wo", wo)):
                    t = wpool.tile([P, DH], BF16, name=f"{nm}_{s}_{k}")
                    nc.sync.dma_start(t[:], src[s, k])
                    wt[nm, s, k] = t
                t = xpool.tile([P, C], BF16, name=f"x_{s}_{k}")
                nc.sync.dma_start(t[:], xt[s, k])
                xts[s, k] = t
        if has_bias:
            bts = {}
            for i in range(3):
                for s in range(2):
                    t = wpool.tile([P, MT], F32, name=f"b_{i}_{s}")
                    nc.sync.dma_start(t[:], bb[i, s])
                    bts[i, s] = t

        for s in range(2):
            for nb in range(NB):
                cols = bass.ts(nb, NBLK)
                us = []
                for m in range(MT):
                    msl = bass.ts(m, P)
                    ph1 = pspool.tile([P, NBLK], F32, name="ph1")
                    for k in range(KT):
                        nc.tensor.matmul(
                            ph1[:],
                            wt["w1", s, k][:, msl],
                            xts[s, k][:, cols],
                            start=(k == 0),
                            stop=(k == KT - 1),
                        )
                    ph2 = pspool.tile([P, NBLK], F32, name="ph2")
                    for k in range(KT):
                        nc.tensor.matmul(
                            ph2[:],
                            wt["w2", s, k][:, msl],
                            xts[s, k][:, cols],
                            start=(k == 0),
                            stop=(k == KT - 1),
                        )
                    sl = spool.tile([P, NBLK], BF16, name="sl")
                    if has_bias:
                        nc.scalar.activation(
                            sl[:], ph1[:], Silu, bias=bts[0, s][:, m : m + 1]
                        )
                        h2 = spool.tile([P, NBLK], F32, name="h2s")
                        nc.scalar.activation(
                            h2[:], ph2[:], Ident, bias=bts[1, s][:, m : m + 1]
                        )
                        h2src = h2
                    else:
                        nc.scalar.activation(sl[:], ph1[:], Silu)
                        h2src = ph2
                    u = upool.tile([P, NBLK], BF16, name=f"u{m}")
                    nc.vector.tensor_mul(u[:], sl[:], h2src[:])
                    us.append(u)
                for m2 in range(MT):
                    m2sl = bass.ts(m2, P)
                    py = pspool.tile([P, NBLK], F32, name="py")
                    for k2 in range(KT):
                        nc.tensor.matmul(
                            py[:],
                            wt["wo", s, k2][:, m2sl],
                            us[k2][:],
                            start=(k2 == 0),
                            stop=(k2 == KT - 1),
                        )
                    yo = ypool.tile([P, NBLK], F32, name="yo")
                    if has_bias:
                        nc.scalar.activation(
                            yo[:], py[:], Ident, bias=bts[2, s][:, m2 : m2 + 1]
                        )
                    else:
                        nc.vector.tensor_copy(yo[:], py[:])
                    nc.sync.dma_start(yt[s, m2, :, cols], yo[:])

    nc.compile()
    return nc


def _get_program(NB, NBLK, KT, MT, DH, has_bias):
    key = (NB, NBLK, KT, MT, DH, has_bias)
    if key not in _PROGRAM_CACHE:
        _PROGRAM_CACHE[key] = _build_program(NB, NBLK, KT, MT, DH, has_bias)
    return _PROGRAM_CACHE[key]


def _route(x2d, w_router, bs_shape):
    """Exactly mirror the reference router; returns (indices[T,2], gates[T,2])."""
    logits = x2d @ w_router.astype(np.float32)  # [T, E]
    noise = _gumbel_noise(tuple(bs_shape) + (w_router.shape[1],)) * NOISE_SCALE
    z = (logits + noise.reshape(logits.shape)).astype(np.float32)
    zmax = z.max(axis=-1, keepdims=True)
    ez = np.exp(z - zmax)
    probs = ez / ez.sum(axis=-1, keepdims=True)
    i1 = np.argmax(probs, axis=-1)
    rows = np.arange(probs.shape[0])
    g1 = probs[rows, i1]
    pm = probs.copy()
    pm[rows, i1] = -np.inf
    i2 = np.argmax(pm, axis=-1)
    g2 = probs[rows, i2]
    idx = np.stack([i1, i2], axis=-1).astype(np.int32)
    gates = np.stack([g1, g2], axis=-1).astype(np.float32)
    return idx, gates


def kernel(x, w_router, W1, b1, W2, b2, Wout, bout, _want_results=False):
    from concourse.bass_utils import run_bass_kernel_spmd

    x = np.asarray(x, dtype=np.float32)
    w_router = np.asarray(w_router, dtype=np.float32)
    W1 = np.asarray(W1, dtype=np.float32)
    W2 = np.asarray(W2, dtype=np.float32)
    Wout = np.asarray(Wout, dtype=np.float32)
    b1 = np.asarray(b1, dtype=np.float32)
    b2 = np.asarray(b2, dtype=np.float32)
    bout = np.asarray(bout, dtype=np.float32)

    B, S, D = x.shape
    E = w_router.shape[1]
    DH = W1.shape[2]
    assert D % P == 0 and DH % P == 0
    KT = D // P
    MT = DH // P
    assert E == 2 * N_CORES, "this kernel hardcodes 2 experts per core"
    T = B * S
    x2d = x.reshape(T, D)

    # ---- router (host) ----
    idx, gates = _route(x2d, w_router, (B, S))

    # ---- expert assignment / capacity ----
    tok_lists = []
    for e in range(E):
        hits = np.where(idx == e)
        tok_lists.append((hits[0], gates[hits[0], hits[1]]))
    counts = np.array([len(t[0]) for t in tok_lists])
    cmax = max(int(counts.max()), 16)
    NB, NBLK = _pick_blocks(cmax)
    C = NB * NBLK

    has_bias = bool(np.any(b1) or np.any(b2) or np.any(bout))
    nc = _get_program(NB, NBLK, KT, MT, DH, has_bias)

    # ---- stage per-core inputs ----
    bf16 = ml_dtypes.bfloat16
    in_maps = []
    for c in range(N_CORES):
        m = {}
        xt = np.zeros((2, KT * P, C), dtype=bf16)
        for s in range(2):
            e = 2 * c + s
            toks = tok_lists[e][0]
            if len(toks):
                xt[s, :, : len(toks)] = x2d[toks].astype(bf16).T
        m["xt"] = np.ascontiguousarray(xt.reshape(2, KT, P, C))
        for nm, W in (("w1", W1), ("w2", W2), ("wo", Wout)):
            m[nm] = np.ascontiguousarray(
                W[2 * c : 2 * c + 2].astype(bf16).reshape(2, KT, P, DH)
            )
        if has_bias:
            bb = np.zeros((3, 2, P, MT), dtype=np.float32)
            for i, bv in enumerate((b1, b2, bout)):
                for s in range(2):
                    bb[i, s] = bv[2 * c + s].reshape(MT, P).T
            m["bb"] = bb
        in_maps.append(m)

    # ---- run on the 8 cores ----
    res = run_bass_kernel_spmd(nc, in_maps, list(range(N_CORES)))

    # ---- combine (host) ----
    out = np.zeros((T, D), dtype=np.float32)
    for c in range(N_CORES):
        yt = res.results[c]["yt"]  # [2, MT, P, C] f32
        for s in range(2):
            e = 2 * c + s
            toks, g = tok_lists[e]
            n = len(toks)
            if n == 0:
                continue
            y = yt[s].reshape(DH, C)[:, :n]  # [D, n]
            out[toks] += g[:, None] * y.T
    out = out.reshape(B, S, D)
    if _want_results:
        return out, res
    return out
